# revision 3
# baseline (speedup 1.0000x reference)
"""MLA attention kernel for Trainium2 (8 NeuronCores, Bass/Tile).

Sharding: 8 cores = 2 batches x 4 kv-head-groups. Core c handles batch
b=c//4 and kv head g=c%4 (query heads 4g..4g+3).

The dominant cost of this problem is host<->device staging, so every
input byte is shipped exactly once in bf16 and redistributed on-device
with collectives:
  * hidden states: each core uploads its (batch, seq-quarter) slice of
    hsT; AllGather over the 4 cores of a batch rebuilds full hsT.
  * per-head-group weights (q proj, o proj, k/v decompress): identical
    on the two cores (g, g+4), so each uploads half and an AllGather
    over pairs [[g, g+4]] restores both halves.
  * fully replicated tensors (kv_compress, causal masks): sharded 8
    ways + AllGather over all cores.
  * o_proj partials: ReduceScatter over each batch group sums the four
    head-group partials on-device; each core downloads only its
    seq-quarter of the result, in bf16.

Host-side algebraic preprocessing (exact, as in the f32 baseline):
  * the reference's apply_rope indexes the rope cache with the HEAD
    axis, so RoPE is a constant per-head rotation folded into
    Wqrope/Wkrope columns;
  * v is zero-padded to 192 dims before out@Wo, so Wo shrinks to the
    128-rows-per-head submatrix;
  * q/k projections merge into single [nope|rope] column blocks.

On-device compute is bf16 with f32 PSUM accumulation: qT/cT
projections, kT/v decompress, causal flash-style attention (scoresT
layout, softmax without max-subtraction - scores are bounded ~|2|,
denominator via ones-matmul), o_proj partial, ReduceScatter.
"""

import os
import sys

import numpy as np
import ml_dtypes

sys.path.insert(0, "/opt/trn_rl_repo")

BF = ml_dtypes.bfloat16

P = 128
B, S, HID = 2, 2048, 2048
H, KV, HD, RD = 16, 4, 128, 64
DF = HD + RD  # 192
CD = 512
NH = H // KV  # heads per core = 4
NK = HID // P  # 16
NS = S // P  # 16
QB = 512
NQ = S // QB  # 4
WQC = NH * HD + NH * RD  # 768 columns of the merged q projection
SCALE = 1.0 / float(np.sqrt(DF))

_NC_CACHE = {}


def build_mla_nc(debug=False):
    import concourse.tile as tile
    from concourse import bacc
    import concourse.mybir as mybir

    F32 = mybir.dt.float32
    BF16 = mybir.dt.bfloat16
    AF = mybir.ActivationFunctionType

    nc = bacc.Bacc(
        "TRN2", target_bir_lowering=False, debug=debug, num_devices=8
    )

    # ---- External I/O (all bf16, each byte shipped once) ----
    hsq = nc.dram_tensor("hsq", [HID, QB], BF16, kind="ExternalInput")
    wqh = nc.dram_tensor("wqh", [HID, WQC // 2], BF16, kind="ExternalInput")
    woh = nc.dram_tensor("woh", [NH * HD // 2, HID], BF16, kind="ExternalInput")
    wkvh = nc.dram_tensor("wkvh", [CD // 2, DF + HD], BF16, kind="ExternalInput")
    wch = nc.dram_tensor("wch", [HID // 8, CD], BF16, kind="ExternalInput")
    mkh = nc.dram_tensor("mkh", [P // 8, NQ * QB], BF16, kind="ExternalInput")
    outq = nc.dram_tensor("outq", [QB, HID], BF16, kind="ExternalOutput")

    # ---- Internal staging (collectives cannot touch IO tensors) ----
    hsq_i = nc.dram_tensor("hsq_i", [HID, QB], BF16)
    wqh_i = nc.dram_tensor("wqh_i", [HID, WQC // 2], BF16)
    woh_i = nc.dram_tensor("woh_i", [NH * HD // 2, HID], BF16)
    wkvh_i = nc.dram_tensor("wkvh_i", [CD // 2, DF + HD], BF16)
    wch_i = nc.dram_tensor("wch_i", [HID // 8, CD], BF16)
    mkh_i = nc.dram_tensor("mkh_i", [P // 8, NQ * QB], BF16)

    # ---- Gathered tensors ----
    hs_g = nc.dram_tensor("hs_g", [NQ * HID, QB], BF16)  # 4 seq quarters
    wq_g = nc.dram_tensor("wq_g", [2 * HID, WQC // 2], BF16)  # 2 col halves
    wo_g = nc.dram_tensor("wo_g", [NH * HD, HID], BF16)
    wkv_g = nc.dram_tensor("wkv_g", [CD, DF + HD], BF16)
    wc_g = nc.dram_tensor("wc_g", [HID, CD], BF16, addr_space="Shared")
    mk_g = nc.dram_tensor("mk_g", [P, NQ * QB], BF16, addr_space="Shared")

    # ---- o_proj partial + reduce-scatter result ----
    part_d = nc.dram_tensor("part_d", [S, HID], BF16)
    rs_d = nc.dram_tensor("rs_d", [QB, HID], BF16)

    GB = [[0, 1, 2, 3], [4, 5, 6, 7]]  # batch groups (seq quarters)
    GP = [[0, 4], [1, 5], [2, 6], [3, 7]]  # weight-sharing pairs
    GA = [[0, 1, 2, 3, 4, 5, 6, 7]]

    def mm(ps, lhsT, rhs, start, stop):
        nc.tensor.matmul(ps, lhsT, rhs, start=start, stop=stop)

    with tile.TileContext(nc) as tc:
        # ---- Stage inputs into internal DRAM, then gather ----
        nc.sync.dma_start(out=hsq_i[:, :], in_=hsq[:, :])
        nc.sync.dma_start(out=wqh_i[:, :], in_=wqh[:, :])
        nc.sync.dma_start(out=woh_i[:, :], in_=woh[:, :])
        nc.sync.dma_start(out=wkvh_i[:, :], in_=wkvh[:, :])
        nc.sync.dma_start(out=wch_i[:, :], in_=wch[:, :])
        nc.sync.dma_start(out=mkh_i[:, :], in_=mkh[:, :])

        cc = nc.gpsimd.collective_compute
        ADD = mybir.AluOpType.add
        NOP = mybir.AluOpType.bypass
        cc("AllGather", NOP, GB, [hsq_i[:].opt()], [hs_g[:].opt()])
        cc("AllGather", NOP, GP, [wqh_i[:].opt()], [wq_g[:].opt()])
        cc("AllGather", NOP, GP, [woh_i[:].opt()], [wo_g[:].opt()])
        cc("AllGather", NOP, GP, [wkvh_i[:].opt()], [wkv_g[:].opt()])
        cc("AllGather", NOP, GA, [wch_i[:].opt()], [wc_g[:].opt()])
        cc("AllGather", NOP, GA, [mkh_i[:].opt()], [mk_g[:].opt()])

        # ---- Long-lived SBUF tiles ----
        with tc.tile_pool(name="qct", bufs=1) as qct, \
             tc.tile_pool(name="ktv", bufs=1) as ktv, \
             tc.tile_pool(name="cons", bufs=1) as cons, \
             tc.tile_pool(name="outn", bufs=1) as outn:
            # q nope: blocks 0-3 (head h); q rope: [64, 4, S]
            qt_n = qct.tile([P, NH, S], BF16)
            qt_r = qct.tile([64, NH, S], BF16)
            ct_sb = qct.tile([P, CD // P, S], BF16)
            kt_n = ktv.tile([P, S], BF16)
            kt_r = ktv.tile([64, S], BF16)
            v_sb = ktv.tile([P, NS, HD], BF16)
            mask_sb = cons.tile([P, NQ * QB], BF16)
            nc.sync.dma_start(out=mask_sb[:], in_=mk_g[:, :])
            ones_k = cons.tile([P, 1], BF16)
            nc.vector.memset(ones_k[:], 1.0)
            ones_b = cons.tile([1, P], F32)
            nc.vector.memset(ones_b[:], 1.0)
            ones_br = cons.tile([1, P], mybir.dt.float32r)
            nc.scalar.activation(ones_br[:], ones_b[:], AF.Copy)
            out_nT = outn.tile([P, NH, S], BF16)

            # ---- Phase A: qT/cT projections ----
            with tc.tile_pool(name="wqp", bufs=1) as wqp, \
                 tc.tile_pool(name="hsp", bufs=2) as hsp, \
                 tc.tile_pool(name="ppA", bufs=4, space="PSUM") as ppA:
                wq_sb = wqp.tile([P, NK, WQC], BF16)
                nc.sync.dma_start(
                    out=wq_sb[:, :, 0:WQC // 2],
                    in_=wq_g[0:HID, :].rearrange("(k p) m -> p k m", p=P),
                )
                nc.sync.dma_start(
                    out=wq_sb[:, :, WQC // 2:WQC],
                    in_=wq_g[HID:2 * HID, :].rearrange(
                        "(k p) m -> p k m", p=P
                    ),
                )
                wc_sb = wqp.tile([P, NK, CD], BF16)
                nc.sync.dma_start(
                    out=wc_sb[:],
                    in_=wc_g[:, :].rearrange("(k p) m -> p k m", p=P),
                )
                for n in range(NQ):
                    hs_t = hsp.tile([P, NK, QB], BF16)
                    nc.sync.dma_start(
                        out=hs_t[:],
                        in_=hs_g[n * HID:(n + 1) * HID, :].rearrange(
                            "(k p) s -> p k s", p=P
                        ),
                    )
                    # q nope: 4 head blocks of 128
                    for m in range(NH):
                        ps = ppA.tile([P, QB], F32)
                        for k in range(NK):
                            mm(
                                ps[:],
                                wq_sb[:, k, m * P:(m + 1) * P],
                                hs_t[:, k, :],
                                k == 0,
                                k == NK - 1,
                            )
                        nc.scalar.activation(
                            qt_n[:, m, n * QB:(n + 1) * QB], ps[:], AF.Copy
                        )
                    # q rope: 4 head blocks of 64
                    for m in range(NH):
                        ps = ppA.tile([P, QB], F32)
                        c0 = NH * HD + m * RD
                        for k in range(NK):
                            mm(
                                ps[:64, :],
                                wq_sb[:, k, c0:c0 + RD],
                                hs_t[:, k, :],
                                k == 0,
                                k == NK - 1,
                            )
                        nc.scalar.activation(
                            qt_r[:, m, n * QB:(n + 1) * QB],
                            ps[:64, :],
                            AF.Copy,
                        )
                    # c latent: 4 blocks of 128
                    for m in range(CD // P):
                        ps = ppA.tile([P, QB], F32)
                        for k in range(NK):
                            mm(
                                ps[:],
                                wc_sb[:, k, m * P:(m + 1) * P],
                                hs_t[:, k, :],
                                k == 0,
                                k == NK - 1,
                            )
                        nc.scalar.activation(
                            ct_sb[:, m, n * QB:(n + 1) * QB], ps[:], AF.Copy
                        )

            # ---- Phase B: kT = Wk_s.T @ cT ; v = cT.T @ Wv_s ----
            with tc.tile_pool(name="wkv", bufs=1) as wkv, \
                 tc.tile_pool(name="ppB", bufs=2, space="PSUM") as ppB:
                wkv_sb = wkv.tile([P, CD // P, DF + HD], BF16)
                nc.sync.dma_start(
                    out=wkv_sb[:],
                    in_=wkv_g[:, :].rearrange("(c p) m -> p c m", p=P),
                )
                NC = CD // P
                for n in range(NQ):
                    psk = ppB.tile([P, QB], F32)
                    for c in range(NC):
                        mm(
                            psk[:],
                            wkv_sb[:, c, 0:HD],
                            ct_sb[:, c, n * QB:(n + 1) * QB],
                            c == 0,
                            c == NC - 1,
                        )
                    nc.scalar.activation(
                        kt_n[:, n * QB:(n + 1) * QB], psk[:], AF.Copy
                    )
                    psr = ppB.tile([P, QB], F32)
                    for c in range(NC):
                        mm(
                            psr[:64, :],
                            wkv_sb[:, c, HD:DF],
                            ct_sb[:, c, n * QB:(n + 1) * QB],
                            c == 0,
                            c == NC - 1,
                        )
                    nc.scalar.activation(
                        kt_r[:, n * QB:(n + 1) * QB], psr[:64, :], AF.Copy
                    )
                for kp in range(NS):
                    psv = ppB.tile([P, HD], F32)
                    for c in range(NC):
                        mm(
                            psv[:],
                            ct_sb[:, c, kp * P:(kp + 1) * P],
                            wkv_sb[:, c, DF:DF + HD],
                            c == 0,
                            c == NC - 1,
                        )
                    nc.scalar.activation(v_sb[:, kp, :], psv[:], AF.Copy)

            # ---- Phase C: causal attention, scoresT layout ----
            with tc.tile_pool(name="expp", bufs=4) as expp, \
                 tc.tile_pool(name="stC", bufs=4) as stC, \
                 tc.tile_pool(name="scps", bufs=2, space="PSUM") as scps, \
                 tc.tile_pool(name="pvps", bufs=2, space="PSUM") as pvps, \
                 tc.tile_pool(name="smps", bufs=2, space="PSUM") as smps, \
                 tc.tile_pool(name="bcps", bufs=2, space="PSUM") as bcps:
                for h in range(NH):
                    for qb in range(NQ):
                        nkb = 4 * (qb + 1)
                        pv = pvps.tile([P, QB], F32)
                        sm = smps.tile([1, QB], F32)
                        for kb in range(nkb):
                            sc = scps.tile([P, QB], F32)
                            mm(
                                sc[:],
                                kt_n[:, kb * P:(kb + 1) * P],
                                qt_n[:, h, qb * QB:(qb + 1) * QB],
                                True,
                                False,
                            )
                            mm(
                                sc[:],
                                kt_r[:, kb * P:(kb + 1) * P],
                                qt_r[:, h, qb * QB:(qb + 1) * QB],
                                False,
                                True,
                            )
                            ex = expp.tile([P, QB], BF16)
                            nc.scalar.activation(
                                ex[:], sc[:], AF.Exp, scale=SCALE
                            )
                            t = kb - 4 * qb
                            if 0 <= t <= 3:
                                nc.vector.tensor_mul(
                                    ex[:],
                                    ex[:],
                                    mask_sb[:, t * QB:(t + 1) * QB],
                                )
                            mm(
                                pv[:], v_sb[:, kb, :], ex[:],
                                kb == 0, kb == nkb - 1,
                            )
                            mm(
                                sm[:], ones_k[:], ex[:],
                                kb == 0, kb == nkb - 1,
                            )
                        rc = stC.tile([1, QB], mybir.dt.float32r, tag="rc")
                        with nc.allow_low_precision(
                            reason="fp32r out is bit-identical to fp32"
                        ):
                            nc.vector.reciprocal(rc[:], sm[:])
                        bc = bcps.tile([P, QB], F32)
                        mm(bc[:], ones_br[:], rc[:], True, True)
                        bcs = stC.tile([P, QB], F32, tag="bcs")
                        nc.scalar.activation(bcs[:], bc[:], AF.Copy)
                        nc.vector.tensor_mul(
                            out_nT[:, h, qb * QB:(qb + 1) * QB],
                            pv[:],
                            bcs[:],
                        )

            # ---- Phase D: partial = out_nope @ Wo_s -> DRAM ----
            with tc.tile_pool(name="wop", bufs=1) as wop, \
                 tc.tile_pool(name="stD", bufs=4) as stD, \
                 tc.tile_pool(name="ppD", bufs=4, space="PSUM") as ppD:
                wo_sb = wop.tile([P, NH, HID], BF16)
                nc.sync.dma_start(
                    out=wo_sb[:],
                    in_=wo_g[:, :].rearrange("(h p) n -> p h n", p=P),
                )
                for sb in range(NS):
                    for nb in range(NQ):
                        ps = ppD.tile([P, QB], F32)
                        for h in range(NH):
                            mm(
                                ps[:],
                                out_nT[:, h, sb * P:(sb + 1) * P],
                                wo_sb[:, h, nb * QB:(nb + 1) * QB],
                                h == 0,
                                h == NH - 1,
                            )
                        st = stD.tile([P, QB], BF16)
                        nc.scalar.activation(st[:], ps[:], AF.Copy)
                        nc.sync.dma_start(
                            out=part_d[
                                sb * P:(sb + 1) * P, nb * QB:(nb + 1) * QB
                            ],
                            in_=st[:],
                        )

        # ---- ReduceScatter partials over the batch group; download ----
        nc.gpsimd.collective_compute(
            "ReduceScatter",
            mybir.AluOpType.add,
            replica_groups=GB,
            ins=[part_d[:].opt()],
            outs=[rs_d[:].opt()],
        )
        nc.sync.dma_start(out=outq[:, :], in_=rs_d[:, :])

    nc.compile()
    return nc


def get_nc(debug=False):
    key = bool(debug)
    if key not in _NC_CACHE:
        _NC_CACHE[key] = build_mla_nc(debug=debug)
    return _NC_CACHE[key]


def _rope_fold(W, n_heads, in_dim):
    """Fold the reference's (head-indexed) RoPE into projection columns."""
    freqs = 1.0 / (10000.0 ** (np.arange(0, RD, 2, dtype=np.float64) / RD))
    t = np.arange(n_heads, dtype=np.float64)
    f = np.outer(t, freqs)  # (n_heads, RD//2)
    cos = np.cos(f)
    sin = np.sin(f)
    W4 = W.astype(np.float64).reshape(in_dim, n_heads, RD // 2, 2)
    e, o = W4[..., 0], W4[..., 1]
    e2 = e * cos[None] - o * sin[None]
    o2 = e * sin[None] + o * cos[None]
    out = np.stack([e2, o2], axis=-1).reshape(in_dim, n_heads, RD)
    return out.astype(np.float32)


def _build_in_maps(hidden_states, Wq, Wkvc, Wkdec, Wvdec, Wqrope, Wkrope, Wo):
    Wqr = _rope_fold(np.asarray(Wqrope, np.float32), H, HID)  # (HID,H,RD)
    Wkr = _rope_fold(np.asarray(Wkrope, np.float32), KV, CD)  # (CD,KV,RD)
    Wqn = np.asarray(Wq, np.float32).reshape(HID, H, HD)
    Wkn = np.asarray(Wkdec, np.float32).reshape(CD, KV, HD)
    Wv4 = np.asarray(Wvdec, np.float32).reshape(CD, KV, HD)
    Wo_eff = np.asarray(Wo, np.float32).reshape(H, DF, HID)[:, :HD, :]

    i = np.arange(P)[:, None, None]
    t = np.arange(NQ)[None, :, None]
    j = np.arange(QB)[None, None, :]
    masks = (i + P * t <= j).astype(BF).reshape(P, NQ * QB)

    Wkvc_f = np.asarray(Wkvc, np.float32)
    hs = np.asarray(hidden_states, np.float32)

    in_maps = []
    for core in range(8):
        b, g = core // 4, core % 4
        half = core // 4  # 0: first half, 1: second half
        # merged q projection for group g, columns:
        # [4 heads x 128 nope | 4 heads x 64 rope] = 768
        wq_full = np.concatenate(
            [
                Wqn[:, 4 * g:4 * g + 4, :].reshape(HID, NH * HD),
                Wqr[:, 4 * g:4 * g + 4, :].reshape(HID, NH * RD),
            ],
            axis=1,
        )  # (HID, 768)
        wq_h = wq_full[:, half * (WQC // 2):(half + 1) * (WQC // 2)]
        # k decompress [nope|rope] + v decompress: (CD, 192+128)
        wkv_full = np.concatenate(
            [Wkn[:, g, :], Wkr[:, g, :], Wv4[:, g, :]], axis=1
        )  # (CD, 320)
        wkv_h = wkv_full[half * (CD // 2):(half + 1) * (CD // 2), :]
        wo_full = Wo_eff[4 * g:4 * g + 4].reshape(NH * HD, HID)
        wo_h = wo_full[
            half * (NH * HD // 2):(half + 1) * (NH * HD // 2), :
        ]
        in_maps.append(
            {
                "hsq": np.ascontiguousarray(
                    hs[b, g * QB:(g + 1) * QB, :].T
                ).astype(BF),
                "wqh": np.ascontiguousarray(wq_h).astype(BF),
                "woh": np.ascontiguousarray(wo_h).astype(BF),
                "wkvh": np.ascontiguousarray(wkv_h).astype(BF),
                "wch": np.ascontiguousarray(
                    Wkvc_f[core * (HID // 8):(core + 1) * (HID // 8), :]
                ).astype(BF),
                "mkh": np.ascontiguousarray(
                    masks[core * (P // 8):(core + 1) * (P // 8), :]
                ),
            }
        )
    return in_maps


def kernel(hidden_states, Wq, Wkvc, Wkdec, Wvdec, Wqrope, Wkrope, Wo):
    from concourse.bass_utils import run_bass_kernel_spmd

    in_maps = _build_in_maps(
        hidden_states, Wq, Wkvc, Wkdec, Wvdec, Wqrope, Wkrope, Wo
    )
    nc = get_nc(debug=False)
    trace = os.environ.get("MLA_TRACE") == "1"
    res = run_bass_kernel_spmd(nc, in_maps, list(range(8)), trace=trace)
    kernel._last_result = res
    out = np.empty((B, S, HID), np.float32)
    for b in range(B):
        for g in range(4):
            out[b, g * QB:(g + 1) * QB, :] = res.results[b * 4 + g][
                "outq"
            ].astype(np.float32)
    return out


# revision 13
# speedup vs baseline: 1.1034x; 1.1034x over previous
"""MLA attention kernel for Trainium2 (8 NeuronCores, Bass/Tile).

Sharding: 8 cores = 2 batches x 4 kv-head-groups. Core c handles batch
b=c//4 and kv head g=c%4 (query heads 4g..4g+3).

The dominant cost of this problem is host<->device staging, so every
input byte is shipped exactly once in bf16 and redistributed on-device
with collectives:
  * hidden states: each core uploads its (batch, seq-quarter) slice of
    hsT; AllGather over the 4 cores of a batch rebuilds full hsT.
  * per-head-group weights (q proj, o proj, k/v decompress): identical
    on the two cores (g, g+4), so each uploads half and an AllGather
    over pairs [[g, g+4]] restores both halves.
  * fully replicated tensors (kv_compress, causal masks): sharded 8
    ways + AllGather over all cores.
  * o_proj partials: ReduceScatter over each batch group sums the four
    head-group partials on-device; each core downloads only its
    seq-quarter of the result, in bf16.

Host-side algebraic preprocessing (exact, as in the f32 baseline):
  * the reference's apply_rope indexes the rope cache with the HEAD
    axis, so RoPE is a constant per-head rotation folded into
    Wqrope/Wkrope columns;
  * v is zero-padded to 192 dims before out@Wo, so Wo shrinks to the
    128-rows-per-head submatrix;
  * q/k projections merge into single [nope|rope] column blocks.

On-device compute is bf16 with f32 PSUM accumulation: qT/cT
projections, kT/v decompress, causal flash-style attention (scoresT
layout, softmax without max-subtraction - scores are bounded ~|2|,
denominator via ones-matmul), o_proj partial, ReduceScatter.
"""

import os
import sys

import numpy as np
import ml_dtypes

sys.path.insert(0, "/opt/trn_rl_repo")

BF = ml_dtypes.bfloat16

P = 128
B, S, HID = 2, 2048, 2048
H, KV, HD, RD = 16, 4, 128, 64
DF = HD + RD  # 192
CD = 512
NH = H // KV  # heads per core = 4
NK = HID // P  # 16
NS = S // P  # 16
QB = 512
NQ = S // QB  # 4
WQC = NH * HD + NH * RD  # 768 columns of the merged q projection
SCALE = 1.0 / float(np.sqrt(DF))

_NC_CACHE = {}


def build_mla_nc(debug=False):
    import concourse.tile as tile
    from concourse import bacc
    import concourse.mybir as mybir

    F32 = mybir.dt.float32
    BF16 = mybir.dt.bfloat16
    AF = mybir.ActivationFunctionType

    nc = bacc.Bacc(
        "TRN2", target_bir_lowering=False, debug=debug, num_devices=8
    )

    # ---- External I/O (all bf16, each byte shipped once) ----
    hsq = nc.dram_tensor("hsq", [HID, QB], BF16, kind="ExternalInput")
    wqh = nc.dram_tensor("wqh", [HID, WQC // 2], BF16, kind="ExternalInput")
    woh = nc.dram_tensor("woh", [NH * HD // 2, HID], BF16, kind="ExternalInput")
    wkvh = nc.dram_tensor("wkvh", [CD // 2, DF + HD], BF16, kind="ExternalInput")
    wch = nc.dram_tensor("wch", [HID // 8, CD], BF16, kind="ExternalInput")
    mkh = nc.dram_tensor("mkh", [P // 8, NQ * QB], BF16, kind="ExternalInput")
    outq = nc.dram_tensor("outq", [QB, HID], BF16, kind="ExternalOutput")

    # ---- Internal staging (collectives cannot touch IO tensors) ----
    hsq_i = nc.dram_tensor("hsq_i", [HID, QB], BF16)
    wqh_i = nc.dram_tensor("wqh_i", [HID, WQC // 2], BF16)
    woh_i = nc.dram_tensor("woh_i", [NH * HD // 2, HID], BF16)
    wkvh_i = nc.dram_tensor("wkvh_i", [CD // 2, DF + HD], BF16)
    wch_i = nc.dram_tensor("wch_i", [HID // 8, CD], BF16)
    mkh_i = nc.dram_tensor("mkh_i", [P // 8, NQ * QB], BF16)

    # ---- Gathered tensors ----
    # hs gather is split into 4 hid-slices so Phase A can stream behind
    # the (serially executing) collectives.
    hs_gs = [
        nc.dram_tensor(f"hs_g{j}", [NQ * (HID // 4), QB], BF16)
        for j in range(4)
    ]
    wq_g = nc.dram_tensor("wq_g", [2 * HID, WQC // 2], BF16)  # 2 col halves
    wo_g = nc.dram_tensor("wo_g", [NH * HD, HID], BF16)
    wkv_g = nc.dram_tensor("wkv_g", [CD, DF + HD], BF16)
    wc_g = nc.dram_tensor("wc_g", [HID, CD], BF16, addr_space="Shared")
    mk_g = nc.dram_tensor("mk_g", [P, NQ * QB], BF16, addr_space="Shared")

    # ---- o_proj partial + reduce-scatter result ----
    part_d = nc.dram_tensor("part_d", [S, HID], BF16)
    rs_d = nc.dram_tensor("rs_d", [QB, HID], BF16)

    GB = [[0, 1, 2, 3], [4, 5, 6, 7]]  # batch groups (seq quarters)
    GP = [[0, 4], [1, 5], [2, 6], [3, 7]]  # weight-sharing pairs
    GA = [[0, 1, 2, 3, 4, 5, 6, 7]]

    def mm(ps, lhsT, rhs, start, stop):
        nc.tensor.matmul(ps, lhsT, rhs, start=start, stop=stop)

    with tile.TileContext(nc) as tc:
        # ---- Stage inputs into internal DRAM, then gather ----
        nc.sync.dma_start(out=hsq_i[:, :], in_=hsq[:, :])
        nc.sync.dma_start(out=wqh_i[:, :], in_=wqh[:, :])
        nc.sync.dma_start(out=woh_i[:, :], in_=woh[:, :])
        nc.sync.dma_start(out=wkvh_i[:, :], in_=wkvh[:, :])
        nc.sync.dma_start(out=wch_i[:, :], in_=wch[:, :])
        nc.sync.dma_start(out=mkh_i[:, :], in_=mkh[:, :])

        # Collectives execute serially; issue in the order compute
        # consumes them: c-projection (wc) first, then hs hid-slices,
        # then q weights, then B/C/D inputs.
        cc = nc.gpsimd.collective_compute
        ADD = mybir.AluOpType.add
        NOP = mybir.AluOpType.bypass
        HQ = HID // 4

        def ag_hs(j):
            cc(
                "AllGather",
                NOP,
                GB,
                [hsq_i[j * HQ:(j + 1) * HQ, :].opt()],
                [hs_gs[j][:].opt()],
            )

        cc("AllGather", NOP, GA, [wch_i[:].opt()], [wc_g[:].opt()])
        ag_hs(0)
        cc("AllGather", NOP, GP, [wqh_i[:].opt()], [wq_g[:].opt()])
        ag_hs(1)
        ag_hs(2)
        ag_hs(3)
        cc("AllGather", NOP, GP, [wkvh_i[:].opt()], [wkv_g[:].opt()])
        cc("AllGather", NOP, GA, [mkh_i[:].opt()], [mk_g[:].opt()])
        cc("AllGather", NOP, GP, [woh_i[:].opt()], [wo_g[:].opt()])

        # ---- Long-lived SBUF tiles ----
        with tc.tile_pool(name="qct", bufs=1) as qct, \
             tc.tile_pool(name="ktv", bufs=1) as ktv, \
             tc.tile_pool(name="cons", bufs=1) as cons, \
             tc.tile_pool(name="outn", bufs=1) as outn:
            # q nope: blocks 0-3 (head h); q rope: [64, 4, S]
            qt_n = qct.tile([P, NH, S], BF16)
            qt_r = qct.tile([64, NH, S], BF16)
            ct_sb = qct.tile([P, CD // P, S], BF16)
            kt_n = ktv.tile([P, S], BF16)
            kt_r = ktv.tile([64, S], BF16)
            v_sb = ktv.tile([P, NS, HD], BF16)
            mask_sb = cons.tile([P, NQ * QB], BF16)
            nc.sync.dma_start(out=mask_sb[:], in_=mk_g[:, :])
            ones_k = cons.tile([P, 1], BF16)
            nc.vector.memset(ones_k[:], 1.0)
            ones_b = cons.tile([1, P], F32)
            nc.vector.memset(ones_b[:], 1.0)
            ones_br = cons.tile([1, P], mybir.dt.float32r)
            nc.scalar.activation(ones_br[:], ones_b[:], AF.Copy)
            out_nT = outn.tile([P, NH, S], BF16)

            # ---- Phase A: qT/cT projections ----
            with tc.tile_pool(name="wqp", bufs=1) as wqp, \
                 tc.tile_pool(name="hsp", bufs=2) as hsp, \
                 tc.tile_pool(name="ppA", bufs=4, space="PSUM") as ppA:
                wq_sb = wqp.tile([P, NK, WQC], BF16)
                nc.sync.dma_start(
                    out=wq_sb[:, :, 0:WQC // 2],
                    in_=wq_g[0:HID, :].rearrange("(k p) m -> p k m", p=P),
                )
                nc.sync.dma_start(
                    out=wq_sb[:, :, WQC // 2:WQC],
                    in_=wq_g[HID:2 * HID, :].rearrange(
                        "(k p) m -> p k m", p=P
                    ),
                )
                wc_sb = wqp.tile([P, NK, CD], BF16)
                nc.sync.dma_start(
                    out=wc_sb[:],
                    in_=wc_g[:, :].rearrange("(k p) m -> p k m", p=P),
                )
                for n in range(NQ):
                    # one tile per hid-slice so matmuls on slice j only
                    # wait for gather/DMA j (tile-granular deps)
                    hs_ts = []
                    for j in range(4):
                        t = hsp.tile([P, 4, QB], BF16, tag=f"hs{j}")
                        nc.sync.dma_start(
                            out=t[:],
                            in_=hs_gs[j][
                                n * (HID // 4):(n + 1) * (HID // 4), :
                            ].rearrange("(k p) s -> p k s", p=P),
                        )
                        hs_ts.append(t)

                    def rhs(k):
                        return hs_ts[k // 4][:, k % 4, :]

                    # c latent first: only needs the small wc gather
                    for m in range(CD // P):
                        ps = ppA.tile([P, QB], F32)
                        for k in range(NK):
                            mm(
                                ps[:],
                                wc_sb[:, k, m * P:(m + 1) * P],
                                rhs(k),
                                k == 0,
                                k == NK - 1,
                            )
                        nc.scalar.activation(
                            ct_sb[:, m, n * QB:(n + 1) * QB], ps[:], AF.Copy
                        )
                    # q nope: 4 head blocks of 128
                    for m in range(NH):
                        ps = ppA.tile([P, QB], F32)
                        for k in range(NK):
                            mm(
                                ps[:],
                                wq_sb[:, k, m * P:(m + 1) * P],
                                rhs(k),
                                k == 0,
                                k == NK - 1,
                            )
                        nc.scalar.activation(
                            qt_n[:, m, n * QB:(n + 1) * QB], ps[:], AF.Copy
                        )
                    # q rope: 4 head blocks of 64
                    for m in range(NH):
                        ps = ppA.tile([P, QB], F32)
                        c0 = NH * HD + m * RD
                        for k in range(NK):
                            mm(
                                ps[:64, :],
                                wq_sb[:, k, c0:c0 + RD],
                                rhs(k),
                                k == 0,
                                k == NK - 1,
                            )
                        nc.scalar.activation(
                            qt_r[:, m, n * QB:(n + 1) * QB],
                            ps[:64, :],
                            AF.Copy,
                        )

            # ---- Phase B: kT = Wk_s.T @ cT ; v = cT.T @ Wv_s ----
            with tc.tile_pool(name="wkv", bufs=1) as wkv, \
                 tc.tile_pool(name="ppB", bufs=2, space="PSUM") as ppB:
                wkv_sb = wkv.tile([P, CD // P, DF + HD], BF16)
                nc.sync.dma_start(
                    out=wkv_sb[:],
                    in_=wkv_g[:, :].rearrange("(c p) m -> p c m", p=P),
                )
                NC = CD // P
                for n in range(NQ):
                    psk = ppB.tile([P, QB], F32)
                    for c in range(NC):
                        mm(
                            psk[:],
                            wkv_sb[:, c, 0:HD],
                            ct_sb[:, c, n * QB:(n + 1) * QB],
                            c == 0,
                            c == NC - 1,
                        )
                    nc.scalar.activation(
                        kt_n[:, n * QB:(n + 1) * QB], psk[:], AF.Copy
                    )
                    psr = ppB.tile([P, QB], F32)
                    for c in range(NC):
                        mm(
                            psr[:64, :],
                            wkv_sb[:, c, HD:DF],
                            ct_sb[:, c, n * QB:(n + 1) * QB],
                            c == 0,
                            c == NC - 1,
                        )
                    nc.scalar.activation(
                        kt_r[:, n * QB:(n + 1) * QB], psr[:64, :], AF.Copy
                    )
                for kp in range(NS):
                    psv = ppB.tile([P, HD], F32)
                    for c in range(NC):
                        mm(
                            psv[:],
                            ct_sb[:, c, kp * P:(kp + 1) * P],
                            wkv_sb[:, c, DF:DF + HD],
                            c == 0,
                            c == NC - 1,
                        )
                    nc.scalar.activation(v_sb[:, kp, :], psv[:], AF.Copy)

            # ---- Phase C+D: attention, o_proj partial, split RS ----
            # Loop qb outer so each seq quarter's o_proj partial and
            # ReduceScatter slice pipeline behind the next quarter's
            # attention.
            with tc.tile_pool(name="expp", bufs=4) as expp, \
                 tc.tile_pool(name="stC", bufs=4) as stC, \
                 tc.tile_pool(name="wop", bufs=1) as wop, \
                 tc.tile_pool(name="stD", bufs=4) as stD, \
                 tc.tile_pool(name="scps", bufs=2, space="PSUM") as scps, \
                 tc.tile_pool(name="pvps", bufs=2, space="PSUM") as pvps, \
                 tc.tile_pool(name="smps", bufs=1, space="PSUM") as smps, \
                 tc.tile_pool(name="bcps", bufs=1, space="PSUM") as bcps, \
                 tc.tile_pool(name="ppD", bufs=2, space="PSUM") as ppD:
                wo_sb = wop.tile([P, NH, HID], BF16)
                nc.sync.dma_start(
                    out=wo_sb[:],
                    in_=wo_g[:, :].rearrange("(h p) n -> p h n", p=P),
                )
                for qb in range(NQ):
                    for h in range(NH):
                        nkb = 4 * (qb + 1)
                        pv = pvps.tile([P, QB], F32)
                        sm = smps.tile([1, QB], F32)
                        for kb in range(nkb):
                            sc = scps.tile([P, QB], F32)
                            mm(
                                sc[:],
                                kt_n[:, kb * P:(kb + 1) * P],
                                qt_n[:, h, qb * QB:(qb + 1) * QB],
                                True,
                                False,
                            )
                            mm(
                                sc[:],
                                kt_r[:, kb * P:(kb + 1) * P],
                                qt_r[:, h, qb * QB:(qb + 1) * QB],
                                False,
                                True,
                            )
                            ex = expp.tile([P, QB], BF16)
                            nc.scalar.activation(
                                ex[:], sc[:], AF.Exp, scale=SCALE
                            )
                            t = kb - 4 * qb
                            if 0 <= t <= 3:
                                nc.vector.tensor_mul(
                                    ex[:],
                                    ex[:],
                                    mask_sb[:, t * QB:(t + 1) * QB],
                                )
                            mm(
                                pv[:], v_sb[:, kb, :], ex[:],
                                kb == 0, kb == nkb - 1,
                            )
                            mm(
                                sm[:], ones_k[:], ex[:],
                                kb == 0, kb == nkb - 1,
                            )
                        rc = stC.tile([1, QB], mybir.dt.float32r, tag="rc")
                        with nc.allow_low_precision(
                            reason="fp32r out is bit-identical to fp32"
                        ):
                            nc.vector.reciprocal(rc[:], sm[:])
                        bc = bcps.tile([P, QB], F32)
                        mm(bc[:], ones_br[:], rc[:], True, True)
                        bcs = stC.tile([P, QB], F32, tag="bcs")
                        nc.scalar.activation(bcs[:], bc[:], AF.Copy)
                        nc.vector.tensor_mul(
                            out_nT[:, h, qb * QB:(qb + 1) * QB],
                            pv[:],
                            bcs[:],
                        )
                    # o_proj partial for seq quarter qb, then its
                    # ReduceScatter slice and download copy.
                    for sb in range(4 * qb, 4 * qb + 4):
                        for nb in range(NQ):
                            ps = ppD.tile([P, QB], F32)
                            for h in range(NH):
                                mm(
                                    ps[:],
                                    out_nT[:, h, sb * P:(sb + 1) * P],
                                    wo_sb[:, h, nb * QB:(nb + 1) * QB],
                                    h == 0,
                                    h == NH - 1,
                                )
                            st = stD.tile([P, QB], BF16)
                            nc.scalar.activation(st[:], ps[:], AF.Copy)
                            nc.sync.dma_start(
                                out=part_d[
                                    sb * P:(sb + 1) * P,
                                    nb * QB:(nb + 1) * QB,
                                ],
                                in_=st[:],
                            )
                    nc.gpsimd.collective_compute(
                        "ReduceScatter",
                        ADD,
                        replica_groups=GB,
                        ins=[part_d[qb * QB:(qb + 1) * QB, :].opt()],
                        outs=[rs_d[qb * P:(qb + 1) * P, :].opt()],
                    )
                    nc.sync.dma_start(
                        out=outq[qb * P:(qb + 1) * P, :],
                        in_=rs_d[qb * P:(qb + 1) * P, :],
                    )

    nc.compile()
    return nc


def get_nc(debug=False):
    key = bool(debug)
    if key not in _NC_CACHE:
        _NC_CACHE[key] = build_mla_nc(debug=debug)
    return _NC_CACHE[key]


def _rope_fold(W, n_heads, in_dim):
    """Fold the reference's (head-indexed) RoPE into projection columns."""
    freqs = 1.0 / (10000.0 ** (np.arange(0, RD, 2, dtype=np.float64) / RD))
    t = np.arange(n_heads, dtype=np.float64)
    f = np.outer(t, freqs)  # (n_heads, RD//2)
    cos = np.cos(f)
    sin = np.sin(f)
    W4 = W.astype(np.float64).reshape(in_dim, n_heads, RD // 2, 2)
    e, o = W4[..., 0], W4[..., 1]
    e2 = e * cos[None] - o * sin[None]
    o2 = e * sin[None] + o * cos[None]
    out = np.stack([e2, o2], axis=-1).reshape(in_dim, n_heads, RD)
    return out.astype(np.float32)


def _build_in_maps(hidden_states, Wq, Wkvc, Wkdec, Wvdec, Wqrope, Wkrope, Wo):
    Wqr = _rope_fold(np.asarray(Wqrope, np.float32), H, HID)  # (HID,H,RD)
    Wkr = _rope_fold(np.asarray(Wkrope, np.float32), KV, CD)  # (CD,KV,RD)
    Wqn = np.asarray(Wq, np.float32).reshape(HID, H, HD)
    Wkn = np.asarray(Wkdec, np.float32).reshape(CD, KV, HD)
    Wv4 = np.asarray(Wvdec, np.float32).reshape(CD, KV, HD)
    Wo_eff = np.asarray(Wo, np.float32).reshape(H, DF, HID)[:, :HD, :]

    i = np.arange(P)[:, None, None]
    t = np.arange(NQ)[None, :, None]
    j = np.arange(QB)[None, None, :]
    masks = (i + P * t <= j).astype(BF).reshape(P, NQ * QB)

    Wkvc_f = np.asarray(Wkvc, np.float32)
    hs = np.asarray(hidden_states, np.float32)

    in_maps = []
    for core in range(8):
        b, g = core // 4, core % 4
        half = core // 4  # 0: first half, 1: second half
        # merged q projection for group g, columns:
        # [4 heads x 128 nope | 4 heads x 64 rope] = 768
        wq_full = np.concatenate(
            [
                Wqn[:, 4 * g:4 * g + 4, :].reshape(HID, NH * HD),
                Wqr[:, 4 * g:4 * g + 4, :].reshape(HID, NH * RD),
            ],
            axis=1,
        )  # (HID, 768)
        wq_h = wq_full[:, half * (WQC // 2):(half + 1) * (WQC // 2)]
        # k decompress [nope|rope] + v decompress: (CD, 192+128)
        wkv_full = np.concatenate(
            [Wkn[:, g, :], Wkr[:, g, :], Wv4[:, g, :]], axis=1
        )  # (CD, 320)
        wkv_h = wkv_full[half * (CD // 2):(half + 1) * (CD // 2), :]
        wo_full = Wo_eff[4 * g:4 * g + 4].reshape(NH * HD, HID)
        wo_h = wo_full[
            half * (NH * HD // 2):(half + 1) * (NH * HD // 2), :
        ]
        in_maps.append(
            {
                "hsq": np.ascontiguousarray(
                    hs[b, g * QB:(g + 1) * QB, :].T
                ).astype(BF),
                "wqh": np.ascontiguousarray(wq_h).astype(BF),
                "woh": np.ascontiguousarray(wo_h).astype(BF),
                "wkvh": np.ascontiguousarray(wkv_h).astype(BF),
                "wch": np.ascontiguousarray(
                    Wkvc_f[core * (HID // 8):(core + 1) * (HID // 8), :]
                ).astype(BF),
                "mkh": np.ascontiguousarray(
                    masks[core * (P // 8):(core + 1) * (P // 8), :]
                ),
            }
        )
    return in_maps


def kernel(hidden_states, Wq, Wkvc, Wkdec, Wvdec, Wqrope, Wkrope, Wo):
    from concourse.bass_utils import run_bass_kernel_spmd

    in_maps = _build_in_maps(
        hidden_states, Wq, Wkvc, Wkdec, Wvdec, Wqrope, Wkrope, Wo
    )
    nc = get_nc(debug=False)
    trace = os.environ.get("MLA_TRACE") == "1"
    res = run_bass_kernel_spmd(nc, in_maps, list(range(8)), trace=trace)
    kernel._last_result = res
    out = np.empty((B, S, HID), np.float32)
    for b in range(B):
        for g in range(4):
            oq = res.results[b * 4 + g]["outq"].astype(np.float32)
            for j in range(NQ):
                r0 = j * QB + g * P
                out[b, r0:r0 + P, :] = oq[j * P:(j + 1) * P]
    return out
